# revision 19
# baseline (speedup 1.0000x reference)
"""Compact-prefix attention (nn_Attention_16234976379516) on 8 TRN2 NeuronCores.

Math per (b, h) pair:
    S = (Q @ K^T) * scale          [T, L]
    S[:, :Lc] += beta              (bias on compacted prefix)
    S = where(mask, S, -inf)       (mask folded into bias host-side)
    O = softmax(S, -1) @ V         [T, D]

Device formulation (transposed scores, no on-chip transposes):
    E^T[l, t] = exp((K Q'^T)[l, t])                     # Q' = Q*scale host-side
    [O*denom | denom] = sum_lc E^T_lc.T @ V'_lc         # PE PSUM accumulation
        where V'[l, :] = e^{bias[l]} * [V[l, :] | 1]    # bias folded into V on
                                                        # host; ones column gives
                                                        # the softmax denominator
    O = (O*denom) * (1/denom)                           # DVE epilogue

The exp over the [L, T] score tile is the ScalarE (ACT) bottleneck, so it is
split across two engines: ACT handles ACT_CHUNKS l-chunks natively; the DVE
handles the rest via two chained custom-DVE ops implementing a degree-5
minimax polynomial for e^(z/8) followed by 3 squarings (max rel err ~5e-4,
bf16-rounding dominated).

Sharding: (B,H) flattened to 128 pairs, 16 per core (pure H/batch split, no
cross-device communication). Host pre-transposes Q/K per head, pre-tiles V
with the ones column and e^bias (beta + mask) folded in, and casts operands
to bf16 (fp32 PSUM accumulation throughout).
"""

import numpy as np

B, H, T, L, LC, D = 4, 32, 256, 4096, 2048, 128
NCORES = 8
G = B * H                  # 128 (b,h) pairs
GPC = G // NCORES          # 16 pairs per core
N_LC = L // 128            # 32 l-chunks of 128
N_TC = T // 128            # 2 t-chunks of 128
SCALE = 1.0 / float(np.sqrt(D))

# l-chunks of scores per PSUM score tile / per exp instruction
ACT_W = 4
N_GROUPS = N_LC // ACT_W   # 8 groups per head
# groups handled by the DVE polynomial exp; the rest go to ScalarE. The DVE
# groups lead each head so the ScalarE groups (processed in order) finish
# last and stage 2 never stalls on the DVE tail.
DVE_GROUPS = (0, 1)
SPLIT_GROUP = 2            # first half DVE, second half ScalarE

# DMA split factors (more concurrent DMA queues in flight)
KT_SPLIT = 4
V1_SPLIT = 4

# Degree-5 minimax poly for e^(z/8) on z in [-8.8, 8.8] (fit offline):
#   q(z) = (z + C_C) + z^2*((W_C + B3*z) + z^2*(B4 + B5*z))
#   e^z ~= (A_C * q(z))^8            (3 squarings on-chip)
C_C = np.float32(8.002612)
W_C = np.float32(0.062401745)
B3 = np.float32(0.0026186928)
B4 = np.float32(8.603248e-05)
B5 = np.float32(1.9488143e-06)
A_C = np.float32(0.12496464)

_NC_CACHE = {}


def _register_dve_exp_ops():
    """Register the two custom-DVE exp ops (idempotent)."""
    from concourse import dve_ops
    from concourse.dve_spec import Spec, Src0, Src1, C0, C1, C2, _has_src1, lower, sq
    from concourse.dve_uop import DveOpSpec

    if "EXP_P5_Q" in dve_ops._SUB_OPCODE_FOR_NAME:
        by_name = {op.name: op for op in dve_ops.OPS}
        return by_name["EXP_P5_Q"], by_name["EXP_P5_SQ"]

    def _ref_q(in0, in1, s0, s1, imm2):
        x = in0.astype(np.float32)
        return ((s1 + x * s0) + (x * x) * (imm2 + x * in1)).astype(np.float32)

    def _ref_sq(in0, in1, s0, s1, imm2):
        x = in0.astype(np.float32)
        p = ((x + s0) + (x * x) * in1.astype(np.float32)) * s1
        for _ in range(3):
            p = p * p
        return p.astype(np.float32)

    # I1 (TTSS): Q(x) = (s1 + s0*x) + x^2*(imm2 + x*src1);  src1 = [P,1] B5
    body_q = (C1 + Src0 * C0) + (Src0 * Src0) * (C2 + Src0 * Src1)
    # I2 (STT): p = ((x + s0) + x^2*Qtile) * s1; out = p^8
    body_sq = sq(sq(sq(((Src0 + C0) + (Src0 * Src0) * Src1) * C1)))

    ops = []
    for name, body, ref in (
        ("EXP_P5_Q", body_q, _ref_q),
        ("EXP_P5_SQ", body_sq, _ref_sq),
    ):
        spec = Spec(body=body, reference=ref)
        row = max(dve_ops._SUB_OPCODE_FOR_NAME.values()) + 1
        uops = lower(spec, ver="v3")
        sha = DveOpSpec(
            name=name, opcode=row, uops=uops, rd1_en=_has_src1(spec)
        ).sha("v3")
        op = dve_ops.DveOp(name, spec, subdim=False, uops_sha={"v3": sha})
        dve_ops.OPS.append(op)
        dve_ops._SUB_OPCODE_FOR_NAME[name] = row
        dve_ops.CUSTOM_DVE_SPECS[name] = spec
        ops.append(op)
    return ops[0], ops[1]


def build_nc(n_heads=GPC, n_lc=N_LC, n_tc=N_TC):
    """Build the single-core Bass program (run SPMD on all 8 cores)."""
    from contextlib import ExitStack

    import concourse.bacc as bacc
    import concourse.mybir as mybir
    import concourse.tile as tile
    from concourse.bass import ts

    op_q, op_sq = _register_dve_exp_ops()

    bf16 = mybir.dt.bfloat16
    f32 = mybir.dt.float32
    L_ = n_lc * 128
    T_ = n_tc * 128

    nc = bacc.Bacc("TRN2", target_bir_lowering=False, debug=False)
    qT_d = nc.dram_tensor("qT", [n_heads, 128, T_], bf16, kind="ExternalInput").ap()
    kT_d = nc.dram_tensor("kT", [n_heads, 128, L_], bf16, kind="ExternalInput").ap()
    # v1[g, p, lc, d]: e^bias[l] * (V row l | 1), l = lc*128 + p
    v1_d = nc.dram_tensor(
        "v1", [n_heads, 128, n_lc, 129], bf16, kind="ExternalInput"
    ).ap()
    out_d = nc.dram_tensor("out", [n_heads, n_tc, 128, 128], bf16, kind="ExternalOutput").ap()

    with tile.TileContext(nc) as tc:
        with ExitStack() as ctx:
            in_pool = ctx.enter_context(tc.tile_pool(name="in_pool", bufs=4))
            e_pool = ctx.enter_context(tc.tile_pool(name="e_pool", bufs=2))
            q_pool = ctx.enter_context(tc.tile_pool(name="q_pool", bufs=2))
            ep_pool = ctx.enter_context(tc.tile_pool(name="ep_pool", bufs=4))
            s_pool = ctx.enter_context(tc.tile_pool(name="s_pool", bufs=3, space="PSUM"))
            o_pool = ctx.enter_context(tc.tile_pool(name="o_pool", bufs=2, space="PSUM"))

            # B5 rides the custom-DVE src1 stream; it must match in0's free
            # length (a [P,1] src1 starves the DVE stream and wedges the
            # engine), so materialize it full-width once.
            c_pool = ctx.enter_context(tc.tile_pool(name="c_pool", bufs=1))
            b5_c = c_pool.tile([128, ACT_W * T_], f32, tag="b5", name="b5_c")
            nc.vector.memset(b5_c, float(B5))

            for g in range(n_heads):
                qT = in_pool.tile([128, T_], bf16, tag="qT", name="qT_sb")
                nc.sync.dma_start(out=qT, in_=qT_d[g])
                kT = in_pool.tile([128, L_], bf16, tag="kT", name="kT_sb")
                for c in range(KT_SPLIT):
                    w = L_ // KT_SPLIT
                    nc.sync.dma_start(
                        out=kT[:, c * w : (c + 1) * w],
                        in_=kT_d[g, :, c * w : (c + 1) * w],
                    )
                v1 = in_pool.tile([128, n_lc, 129], bf16, tag="v1", name="v1_sb")
                for c in range(V1_SPLIT):
                    w = n_lc // V1_SPLIT
                    nc.gpsimd.dma_start(
                        out=v1[:, c * w : (c + 1) * w, :],
                        in_=v1_d[g, :, c * w : (c + 1) * w, :],
                    )

                # Stage 1: E^T[l, t] = exp((K Q'^T)[l, t]), bf16 (scale folded
                # into Q' host-side). ACT_W score matmuls land in one 2-bank
                # PSUM tile; groups < ACT_GROUPS take one wide ScalarE Exp,
                # the rest go through the 2-op custom-DVE polynomial exp.
                e = e_pool.tile([128, n_lc, T_], bf16, tag="e", name="e_sb")
                for a in range(N_GROUPS):
                    s = s_pool.tile([128, ACT_W, T_], f32, tag="s", name="s_ps")
                    for j in range(ACT_W):
                        lc = a * ACT_W + j
                        nc.tensor.matmul(
                            s[:, j, :],
                            lhsT=kT[:, ts(lc, 128)],
                            rhs=qT,
                            start=True,
                            stop=True,
                        )
                    def dve_exp(s_ap, e_ap, width):
                        qt = q_pool.tile([128, ACT_W, T_], f32, tag="qp", name="qp_sb")
                        qt_ap = qt[:, :width, :]
                        nc.vector._custom_dve(
                            op_q, out=qt_ap, in0=s_ap, in1=b5_c[:, : width * T_],
                            s0=float(B3), s1=float(W_C), imm2=float(B4),
                        )
                        nc.vector._custom_dve(
                            op_sq, out=e_ap, in0=s_ap, in1=qt_ap,
                            s0=float(C_C), s1=float(A_C),
                        )

                    def act_exp(s_ap, e_ap):
                        nc.scalar.activation(
                            out=e_ap,
                            in_=s_ap,
                            func=mybir.ActivationFunctionType.Exp,
                            scale=1.0,
                        )

                    lo = a * ACT_W
                    eout = e[:, lo : lo + ACT_W, :]
                    if a in DVE_GROUPS:
                        dve_exp(s, eout, ACT_W)
                    elif a == SPLIT_GROUP:
                        h = ACT_W // 2
                        dve_exp(s[:, :h, :], e[:, lo : lo + h, :], h)
                        act_exp(s[:, h:, :], e[:, lo + h : lo + ACT_W, :])
                    else:
                        act_exp(s, eout)

                # Stage 2: accumulate [O*denom | denom] over l-chunks. The two
                # t-chunk accumulators MUST be separate PSUM tiles: start=True
                # resets the whole bank, so sharing one corrupts the other.
                os_ = []
                for tci in range(n_tc):
                    os_.append(o_pool.tile([128, 129], f32, tag="o", name="o_ps"))
                for lc in range(n_lc):
                    for tci in range(n_tc):
                        nc.tensor.matmul(
                            os_[tci],
                            lhsT=e[:, lc, ts(tci, 128)],
                            rhs=v1[:, lc, :],
                            start=(lc == 0),
                            stop=(lc == n_lc - 1),
                        )

                # Epilogue: O = (O*denom) / denom. Reciprocal on DVE; the
                # scaling multiply is a ScalarE copy-with-scale (GpSimd cannot
                # read PSUM). bf16 out halves the writeback DMA.
                for tci in range(n_tc):
                    recip = ep_pool.tile([128, 1], f32, tag="recip", name="recip_sb")
                    nc.vector.reciprocal(recip, os_[tci][:, 128:129])
                    ob = ep_pool.tile([128, 128], bf16, tag="ob", name="ob_sb")
                    nc.scalar.activation(
                        out=ob,
                        in_=os_[tci][:, 0:128],
                        func=mybir.ActivationFunctionType.Copy,
                        scale=recip,
                    )
                    nc.sync.dma_start(out=out_d[g, tci], in_=ob)

    nc.compile()
    return nc


def make_core_inputs(q, k, v, beta, attn_mask):
    """Host prep: fold mask+beta into bias, transpose/tile/cast, shard 8 ways.

    Returns list of 8 in_maps (one per core)."""
    import ml_dtypes

    bf16 = ml_dtypes.bfloat16

    qf = np.ascontiguousarray(q, np.float32).reshape(G, T, D) * np.float32(SCALE)
    kf = np.ascontiguousarray(k, np.float32).reshape(G, L, D)
    vf = np.ascontiguousarray(v, np.float32).reshape(G, L, D)

    bias = np.zeros((G, L), np.float32)
    bias[:, :LC] = np.asarray(beta, np.float32).reshape(G, LC)
    mask = np.asarray(attn_mask).reshape(G, L)
    # exp(s + b) = exp(s) * e^b: fold e^bias into the [V | 1] operand so the
    # device exp needs no per-partition bias (enables wide ACT tiles). A
    # masked-out l gets e^-inf = 0, zeroing its numerator+denominator terms.
    ebias = np.where(mask, np.exp(bias), np.float32(0.0))

    in_maps = []
    for i in range(NCORES):
        sl = slice(i * GPC, (i + 1) * GPC)
        qT = np.ascontiguousarray(qf[sl].transpose(0, 2, 1)).astype(bf16)
        kT = np.ascontiguousarray(kf[sl].transpose(0, 2, 1)).astype(bf16)
        v1 = np.empty((GPC, L, D + 1), np.float32)
        v1[..., :D] = vf[sl]
        v1[..., D] = 1.0
        v1 *= ebias[sl, :, None]
        v1 = v1.reshape(GPC, N_LC, 128, D + 1).transpose(0, 2, 1, 3)
        in_maps.append(
            {"qT": qT, "kT": kT, "v1": np.ascontiguousarray(v1.astype(bf16))}
        )
    return in_maps


def run_spmd(in_maps, trace=False):
    from concourse import bass_utils

    if "nc" not in _NC_CACHE:
        _NC_CACHE["nc"] = build_nc()
    nc = _NC_CACHE["nc"]
    return bass_utils.run_bass_kernel_spmd(
        nc, in_maps, core_ids=list(range(NCORES)), trace=trace
    )


def kernel(q, k, v, beta, attn_mask):
    res = run_spmd(make_core_inputs(q, k, v, beta, attn_mask))
    out = np.empty((G, T, D), np.float32)
    for i in range(NCORES):
        out[i * GPC : (i + 1) * GPC] = res.results[i]["out"].reshape(GPC, T, D)
    return out.reshape(B, H, T, D)
